# revision 20
# baseline (speedup 1.0000x reference)
"""Single-head causal attention (B=4, L=2048, D=1024) on 8 TRN2 NeuronCores.

Sharding: data-parallel over batch (4) x interleaved query-tile halves (2).
Core 2b+h handles batch b and global q-tiles {h, h+2, ..., h+14}.

Padded keys (~half of all keys) are compressed out on the host: the kernel
only projects/scores the kept keys (capacity CT*128 slots, CT derived from
the actual inputs at compile time with a recompile guard). Masked logits map
to E = exp(-512) = exact 0, so skipped/dummy slots contribute nothing. A
virtual key in slot 0 (kT column = 0, value row = mean of ALL value rows,
cmp scale 0.94140625 -> E = exp(-30) for every query) reproduces the
reference's fully-masked-row semantics (uniform average over all 2048 keys)
exactly while perturbing normal rows by ~1e-13.

The V projection is split by column halves across each core pair and the
peer half is fetched with a single pair-local AllGather. The collective
path has a fixed ~35us startup barrier plus ~10us trigger latency, so the
schedule keeps it OFF the critical path: AV is split into an A-phase (own
V columns + softmax normalizer, fully local, output half stored early) and
a B-phase (peer V columns) that runs last, ~20us after the worst-case
AllGather landing. The peer slot of the AllGather output is selected with
two cond-predicated DMAs keyed on a host-provided rank flag (the skipped
DMA still bumps its semaphore, so SPMD dependencies stay uniform). Output
columns are written [own|peer] and the host unshuffles them per core.

The Q/K path runs fp8e4m3 with DoubleRow matmuls; V/E/AV stay bf16. wq/wk
are pre-scaled x256 on host; the 2^16 compensation folds into the exp
scale (2^-21). Scores are computed TRANSPOSED: sT[k(128 part), q(free)] so
the mask is a per-partition tensor_scalar operand and the softmax
normalizer Z comes from a ones-column matmul - no partition reductions
anywhere. masked_fill:
    cmp[k,q] = (iota_q >= thresh[k]) * kscale[k]
    E        = exp(((s' + 2^30) * cmp) * 2^-21 - 512)
             = exp(s_raw/32) kept, 0 masked (exp(-512) underflows),
               exp(-30) virtual.

Static causal pruning is data-specialized: scores for k-tile kt start at
q-tile jl0[kt]; AV for q-tile jl accumulates nkt[jl] k-tiles; both derived
from the compressed key positions (min over batches/halves, so the shared
SPMD program covers every core; per-core dead regions fall out as E = 0).
Input loads ride the Activation HWDGE queue so the in-order sync queue
only carries staging/assembly/output DMAs in timeline order.
"""
import sys

if "/opt/trn_rl_repo" not in sys.path:
    sys.path.insert(0, "/opt/trn_rl_repo")

import numpy as np
import ml_dtypes

import concourse.bass as bass
import concourse.mybir as mybir
from concourse import bacc, tile
from concourse import bass_utils

F32 = mybir.dt.float32
I32 = mybir.dt.int32
FP8 = mybir.dt.float8e4
FP8NP = ml_dtypes.float8_e4m3
BF16 = mybir.dt.bfloat16
BF16NP = ml_dtypes.bfloat16

B, L, D = 4, 2048, 1024
NQ = L // 2          # queries per core
NMC = D // 128       # 8 contraction/model chunks
NQT = NQ // 128      # 8 q-tiles per core

SW = 256.0           # host pre-scale for wq, wk (fp8 range)
C0 = float(2 ** 30)  # additive pre-mask constant; C0 * 2^-21 = 512
GAMMA = 1.0 - 30.0 / 512.0  # virtual-key cmp scale -> E = exp(-30)
DR = mybir.MatmulPerfMode.DoubleRow

_NC_CACHE = None
_SPEC_CACHE = None


def _make_spec(pad_mask):
    """Static program parameters derived from the pad mask."""
    pad = np.asarray(pad_mask)
    kept = [np.flatnonzero(~pad[b]) for b in range(B)]
    maxk = max(len(k) for k in kept) + 1          # +1 virtual slot
    ct = (maxk + 127) // 128
    # min over batches of the original position of each tile's first slot
    # (virtual = -inf, dummies = +inf)
    minpos = []
    for kt in range(ct):
        m = np.inf
        for b in range(B):
            slot = kt * 128
            if slot == 0:
                m = -np.inf
            elif slot <= len(kept[b]):
                m = min(m, float(kept[b][slot - 1]))
        minpos.append(m)
    jl0 = []
    for kt in range(ct):
        first = NQT
        for h in (0, 1):
            for jl in range(NQT):
                if 128 * (2 * jl + h) + 127 >= minpos[kt]:
                    first = min(first, jl)
                    break
        jl0.append(first)
    nkt = []
    for jl in range(NQT):
        last = 0
        for kt in range(ct):
            if minpos[kt] <= 128 * (2 * jl + 1) + 127:
                last = kt
        nkt.append(last + 1)
    return (ct, tuple(jl0), tuple(nkt))


def _build_nc(spec):
    ct, jl0, nkt = spec
    nck = ct * 128       # compressed key slots
    nc = bacc.Bacc(None, target_bir_lowering=False)

    xk_d = nc.dram_tensor("xk", [128, NMC, nck], FP8, kind="ExternalInput")
    xt16_d = nc.dram_tensor("xt16", [128, nck // 128, NMC, 128], BF16,
                            kind="ExternalInput")
    xq_d = nc.dram_tensor("xq", [128, NMC, NQ], FP8, kind="ExternalInput")
    wq_d = nc.dram_tensor("wq", [128, NMC, D], FP8, kind="ExternalInput")
    wk_d = nc.dram_tensor("wk", [128, NMC, NMC, 128], FP8, kind="ExternalInput")
    wv_d = nc.dram_tensor("wv", [128, NMC, 512], BF16, kind="ExternalInput")
    ksc_d = nc.dram_tensor("ksc", [128, ct], F32, kind="ExternalInput")
    thr_d = nc.dram_tensor("thr", [128, ct], F32, kind="ExternalInput")
    hsel_d = nc.dram_tensor("hsel", [128, 2], I32, kind="ExternalInput")
    out_d = nc.dram_tensor("out", [NQ, D], F32, kind="ExternalOutput")

    AL = mybir.AluOpType
    AF = mybir.ActivationFunctionType

    with tile.TileContext(nc) as tc:
        with (
            tc.tile_pool(name="c", bufs=1) as cpool,
            tc.tile_pool(name="sh", bufs=1) as spool,
            tc.tile_pool(name="wk_", bufs=3) as wpool,
            tc.tile_pool(name="pp", bufs=4, space="PSUM") as pp,
            tc.tile_pool(name="ppo", bufs=3, space="PSUM") as ppo,
            tc.tile_pool(name="ppz", bufs=1, space="PSUM") as ppz,
            tc.tile_pool(name="dr", bufs=1, space="DRAM") as drpool,
        ):
            # persistent tiles; xt16 and E share one slot (disjoint lifetimes)
            xk_sb = cpool.tile([128, NMC, nck], FP8, name="xk_sb")
            vo_sb = cpool.tile([128, ct, 513], BF16, name="vo_sb")
            vp_sb = cpool.tile([128, ct, 512], BF16, name="vp_sb")
            g2_in = drpool.tile([128, ct, 512], BF16, name="g2_in")
            g2_out = drpool.tile([2, 128, ct, 512], BF16, name="g2_out")
            # xt16 is host-laid-out [128, kt, d, 128]: per-k-tile dense
            # chunks load fast and stream under the V-own loop
            xt16_sb = spool.tile([128, nck // 128, NMC, 128], BF16,
                                 name="xt16_sb", tag="big")
            # wk is host-laid-out [128, mi, d, 128] so it loads as one
            # dense DMA (strided mi-column loads ran at ~100 GB/s and
            # stalled the kT start by ~8us)
            wk_sb = cpool.tile([128, NMC, NMC, 128], FP8, name="wk_sb")
            wv_sb = cpool.tile([128, NMC, 512], BF16, name="wv_sb")
            wq_sb = cpool.tile([128, NMC, D], FP8, name="wq_sb")
            xq_sb = cpool.tile([128, NMC, NQ], FP8, name="xq_sb")
            ksc_sb = cpool.tile([128, ct], F32, name="ksc_sb")
            thr_sb = cpool.tile([128, ct], F32, name="thr_sb")
            hsel_sb = cpool.tile([128, 2], I32, name="hsel_sb")
            kT_sb = cpool.tile([128, NMC, nck], FP8, name="kT_sb")
            qT_sb = cpool.tile([128, NMC, NQ], FP8, name="qT_sb")
            rec_sb = cpool.tile([128, NQT], F32, name="rec_sb")
            iota_sb = cpool.tile([128, NQ], F32, name="iota_sb")
            bias_sb = cpool.tile([128, 1], F32, name="bias_sb")

            # Three DMA queues, matched to consumers so no in-order queue
            # ever holds a later-needed transfer behind an earlier-blocked
            # one: scalar (Activation HWDGE) carries the V-path inputs it
            # needs first plus the psum evacuations and output stores;
            # gpsimd carries the K/Q-path inputs (needed from ~31us); sync
            # carries only collective staging and the predicated peer fetch.
            c2 = min(2, ct)
            c6 = min(6, ct)
            nc.scalar.dma_start(wv_sb[:, 0:4], wv_d[:, 0:4])
            nc.scalar.dma_start(xt16_sb[:, 0:c2], xt16_d[:, 0:c2])
            nc.scalar.dma_start(wv_sb[:, 4:8], wv_d[:, 4:8])
            nc.scalar.dma_start(xt16_sb[:, c2:c6], xt16_d[:, c2:c6])
            nc.scalar.dma_start(xt16_sb[:, c6:], xt16_d[:, c6:])
            nc.scalar.dma_start(wk_sb[:, 0:1], wk_d[:, 0:1])
            nc.scalar.dma_start(xk_sb[:], xk_d[:])
            nc.scalar.dma_start(wk_sb[:, 1:NMC], wk_d[:, 1:NMC])
            nc.scalar.dma_start(wq_sb[:], wq_d[:])
            nc.scalar.dma_start(xq_sb[:], xq_d[:])
            nc.scalar.dma_start(ksc_sb[:], ksc_d[:])
            nc.scalar.dma_start(thr_sb[:], thr_d[:])
            nc.scalar.dma_start(hsel_sb[:], hsel_d[:])

            # local q column f (= 128*jl + fi) maps to global q-tile 2*jl + h;
            # iota encodes q_glob - 128*h = 256*jl + fi; thresh data absorbs h.
            nc.gpsimd.iota(
                out=iota_sb[:].rearrange("p (j f) -> p j f", f=128),
                pattern=[[256, NQT], [1, 128]], base=0, channel_multiplier=0,
                allow_small_or_imprecise_dtypes=True,
            )

            # PE clock warmup: the HAM gate holds the PE at low clock until it
            # sees a few us of sustained activity. Run junk matmuls on a
            # memset tile during the initial DMA wait (PE is idle anyway) so
            # the real projections start at full clock. warm_sb's memset goes
            # first so the warmup isn't gated on the other memsets.
            warm_sb = cpool.tile([128, 128], BF16, name="warm_sb")
            nc.vector.memset(warm_sb[:], 0.0)
            nc.vector.memset(bias_sb[:], -512.0)
            nc.vector.memset(vo_sb[:, :, 512:513], 1.0)
            ps_w = pp.tile([128, 512], F32, name="ps")
            for wi in range(40):
                nc.tensor.matmul(
                    ps_w[:, 0:128], lhsT=warm_sb[:], rhs=warm_sb[:],
                    start=(wi == 0), stop=(wi == 39),
                )

            # ---- Phase 1b: V-own[tok, mo] = x_c @ wv_own in bf16 over this
            # core's 512-column half (value path stays high precision). The
            # peer half arrives via one pair-local AllGather, consumed only
            # by the late B-phase of AV. ----
            for kt in range(ct):
                ps = pp.tile([128, 512], F32, name="ps")
                for d in range(NMC):
                    nc.tensor.matmul(
                        ps[:],
                        lhsT=xt16_sb[:, kt, d],
                        rhs=wv_sb[:, d],
                        start=(d == 0), stop=(d == NMC - 1),
                    )
                nc.scalar.copy(vo_sb[:, kt, 0:512], ps[:])
                nc.sync.dma_start(g2_in[:, kt], vo_sb[:, kt, 0:512])
            nc.gpsimd.collective_compute(
                "AllGather", AL.bypass,
                replica_groups=[[0, 1], [2, 3], [4, 5], [6, 7]],
                ins=[g2_in[:]], outs=[g2_out[:]],
            )
            # rank-predicated peer-half fetch: exactly one of these runs on a
            # given core; the skipped one still bumps its semaphore so the
            # B-phase dependency is uniform across the SPMD pair.
            h0_reg = nc.values_load(hsel_sb[0:1, 0:1],
                                    engines=[mybir.EngineType.SP],
                                    min_val=0, max_val=1,
                                    skip_runtime_bounds_check=True)
            h1_reg = nc.values_load(hsel_sb[0:1, 1:2],
                                    engines=[mybir.EngineType.SP],
                                    min_val=0, max_val=1,
                                    skip_runtime_bounds_check=True)
            nc.sync.dma_start(vp_sb[:], g2_out[1], cond=h0_reg)
            nc.sync.dma_start(vp_sb[:], g2_out[0], cond=h1_reg)

            # ---- Phase 1a: kT[m, tok] = wk.T @ x_c over all compressed
            # keys (duplicated per pair: the collective path is too slow and
            # variable to gate the scores phase on a kT exchange). f-outer so
            # the first pass only needs the first xk chunk in SBUF. ----
            for mi in range(NMC):
                f = 0
                while f < nck:
                    w = min(512, nck - f)
                    ps = pp.tile([128, 512], F32, name="ps")
                    for d in range(0, NMC, 2):
                        nc.tensor.matmul(
                            ps[:, 0:w],
                            lhsT=wk_sb[:, mi, d : d + 2, :],
                            rhs=xk_sb[:, d : d + 2, f : f + w],
                            start=(d == 0), stop=(d == NMC - 2), perf_mode=DR,
                        )
                    nc.scalar.copy(kT_sb[:, mi, f : f + w], ps[:, 0:w])
                    f += w

            # ---- Phase 1c: qT[m, q] = wq.T @ xq ----
            for mi in range(NMC):
                ps0 = pp.tile([128, 512], F32, name="ps")
                ps1 = pp.tile([128, 512], F32, name="ps")
                for d in range(0, NMC, 2):
                    for qb, psx in ((0, ps0), (1, ps1)):
                        nc.tensor.matmul(
                            psx[:],
                            lhsT=wq_sb[:, d : d + 2, mi * 128 : (mi + 1) * 128],
                            rhs=xq_sb[:, d : d + 2, qb * 512 : (qb + 1) * 512],
                            start=(d == 0), stop=(d == NMC - 2), perf_mode=DR,
                        )
                for qb, psx in ((0, ps0), (1, ps1)):
                    nc.scalar.copy(qT_sb[:, mi, qb * 512 : (qb + 1) * 512], psx[:])

            # ---- Phase 2: scores (transposed) + mask + exp, per k-tile ----
            E_sb = spool.tile([128, ct, NQ], BF16, name="E_sb", tag="big")
            for kt in range(ct):
                if jl0[kt] >= NQT:
                    continue
                f0 = jl0[kt] * 128
                cmp = wpool.tile([128, NQ], F32, name="cmp", bufs=2)
                nc.vector.tensor_scalar(
                    out=cmp[:, f0:], in0=iota_sb[:, f0:],
                    scalar1=thr_sb[:, kt : kt + 1], scalar2=ksc_sb[:, kt : kt + 1],
                    op0=AL.is_ge, op1=AL.mult,
                )
                s_sb = wpool.tile([128, NQ], F32, name="s_sb", bufs=3)
                f = f0
                while f < NQ:
                    w = min(512, NQ - f)
                    ps = pp.tile([128, 512], F32, name="ps")
                    for m in range(0, NMC, 2):
                        nc.tensor.matmul(
                            ps[:, 0:w],
                            lhsT=kT_sb[:, m : m + 2, kt * 128 : (kt + 1) * 128],
                            rhs=qT_sb[:, m : m + 2, f : f + w],
                            start=(m == 0), stop=(m == NMC - 2), perf_mode=DR,
                        )
                    nc.vector.scalar_tensor_tensor(
                        out=s_sb[:, f : f + w], in0=ps[:, 0:w],
                        scalar=C0,
                        in1=cmp[:, f : f + w],
                        op0=AL.add, op1=AL.mult,
                    )
                    f += w
                nc.scalar.activation(
                    out=E_sb[:, kt, f0:], in_=s_sb[:, f0:],
                    func=AF.Exp, bias=bias_sb[:], scale=2.0 ** -21,
                )

            # ---- Phase 3A: AV over own V columns + normalizer, per q-tile.
            # Fully local: vo_sb col 512 is the ones column for Z. Own-half
            # output stores stream out while scores/B-phase still run. ----
            for jl in range(NQT):
                n = nkt[jl]
                po = ppo.tile([128, 512], F32, name="po")
                pz = ppz.tile([128, 1], F32, name="pz")
                for kta in range(n):
                    lhsT = E_sb[:, kta, jl * 128 : (jl + 1) * 128]
                    nc.tensor.matmul(po[:], lhsT=lhsT,
                                     rhs=vo_sb[:, kta, 0:512],
                                     start=(kta == 0), stop=(kta == n - 1))
                    nc.tensor.matmul(pz[:], lhsT=lhsT,
                                     rhs=vo_sb[:, kta, 512:513],
                                     start=(kta == 0), stop=(kta == n - 1))
                nc.vector.reciprocal(rec_sb[:, jl : jl + 1], pz[:])
                oa = wpool.tile([128, 512], F32, name="oa", bufs=3)
                nc.vector.tensor_scalar(
                    out=oa[:], in0=po[:], scalar1=rec_sb[:, jl : jl + 1],
                    scalar2=None, op0=AL.mult,
                )
                nc.scalar.dma_start(out_d[jl * 128 : (jl + 1) * 128, 0:512],
                                    oa[:])

            # clock-keeper: if the B-phase briefly stalls on the AllGather,
            # junk matmuls keep the HAM gate from halving the engine clock
            ps_f = pp.tile([128, 512], F32, name="ps")
            for wi in range(24):
                nc.tensor.matmul(
                    ps_f[:, 0:128], lhsT=warm_sb[:], rhs=warm_sb[:],
                    start=(wi == 0), stop=(wi == 23),
                )

            # ---- Phase 3B: AV over peer V columns (waits on the AllGather
            # + predicated fetch, which by now have long landed) ----
            for jl in range(NQT):
                n = nkt[jl]
                po = ppo.tile([128, 512], F32, name="po")
                for kta in range(n):
                    lhsT = E_sb[:, kta, jl * 128 : (jl + 1) * 128]
                    nc.tensor.matmul(po[:], lhsT=lhsT,
                                     rhs=vp_sb[:, kta],
                                     start=(kta == 0), stop=(kta == n - 1))
                ob = wpool.tile([128, 512], F32, name="ob", bufs=3)
                nc.vector.tensor_scalar(
                    out=ob[:], in0=po[:], scalar1=rec_sb[:, jl : jl + 1],
                    scalar2=None, op0=AL.mult,
                )
                nc.scalar.dma_start(out_d[jl * 128 : (jl + 1) * 128, 512:1024],
                                    ob[:])

    nc.compile()
    return nc


def _chunked(a):
    """[C*128, N] -> [128, C, N] contiguous."""
    c = a.shape[0] // 128
    return np.ascontiguousarray(a.reshape(c, 128, *a.shape[1:]).transpose(1, 0, 2))


def _qsel(h):
    """Global query rows handled by half h: interleaved 128-row q-tiles."""
    return np.concatenate(
        [np.arange(128 * (2 * jl + h), 128 * (2 * jl + h) + 128) for jl in range(NQT)]
    )


def build_in_maps(inputs, spec=None):
    x = np.asarray(inputs["x"], dtype=np.float32)
    pad = np.asarray(inputs["pad_mask"])
    if spec is None:
        spec = _make_spec(pad)
    ct = spec[0]
    nck = ct * 128
    wq_h = _chunked(np.asarray(inputs["wq"], dtype=np.float32) * SW).astype(FP8NP)
    wk_c = _chunked(np.asarray(inputs["wk"], dtype=np.float32) * SW)  # [128,d,1024]
    wk_h = np.ascontiguousarray(
        wk_c.reshape(128, NMC, NMC, 128).transpose(0, 2, 1, 3)
    ).astype(FP8NP)                                   # [128, mi, d, 128] dense
    wv_f = np.asarray(inputs["wv"], dtype=np.float32)

    in_maps = []
    for c in range(8):
        b, h = divmod(c, 2)
        kept = np.flatnonzero(~pad[b])
        nk = len(kept)
        # compressed x: slot 0 = virtual key (kT col 0, value row = mean x)
        xc = np.zeros((nck, D), np.float32)
        xc[1 : 1 + nk] = x[b, kept]
        xcv = xc.copy()
        xcv[0] = x[b].mean(axis=0)
        thr = np.full(nck, 1e9, np.float32)
        thr[0] = -1e9
        thr[1 : 1 + nk] = kept.astype(np.float32) - 128.0 * h
        ksc = np.ones(nck, np.float32)
        ksc[0] = GAMMA
        hsel = np.zeros((128, 2), np.int32)
        hsel[:, h] = 1    # col0 = is-rank-0, col1 = is-rank-1

        qsel = _qsel(h)
        xkb = _chunked(xc.T).astype(FP8NP)                   # [128, 8, nck]
        xt_c = _chunked(xcv.T)                               # [128, d, nck]
        xtb16 = np.ascontiguousarray(
            xt_c.reshape(128, NMC, nck // 128, 128).transpose(0, 2, 1, 3)
        ).astype(BF16NP)                                     # [128, kt, d, 128]
        xqb = _chunked(x[b, qsel, :].T).astype(FP8NP)        # [128, 8, 1024]
        wvb = _chunked(wv_f[:, h * 512 : (h + 1) * 512]).astype(BF16NP)
        in_maps.append({
            "xk": xkb, "xt16": xtb16, "xq": xqb, "wq": wq_h, "wk": wk_h,
            "wv": wvb, "hsel": hsel,
            "ksc": np.ascontiguousarray(ksc.reshape(ct, 128).T),
            "thr": np.ascontiguousarray(thr.reshape(ct, 128).T),
        })
    return in_maps


def _ensure_compiled(inputs):
    global _NC_CACHE, _SPEC_CACHE
    spec = _make_spec(np.asarray(inputs["pad_mask"]))
    if _NC_CACHE is None or _SPEC_CACHE != spec:
        _NC_CACHE = _build_nc(spec)
        _SPEC_CACHE = spec
    return _NC_CACHE, spec


def kernel(**inputs):
    nc, spec = _ensure_compiled(inputs)
    in_maps = build_in_maps(inputs, spec)
    res = bass_utils.run_bass_kernel_spmd(nc, in_maps, core_ids=list(range(8)))
    out = np.empty((B, L, D), dtype=np.float32)
    for b in range(B):
        for h in range(2):
            r = res.results[2 * b + h]["out"]
            # device wrote [own|peer] column halves; own = global cols
            # [512h, 512h+512)
            out[b, _qsel(h), 512 * h : 512 * h + 512] = r[:, 0:512]
            out[b, _qsel(h), 512 * (1 - h) : 512 * (1 - h) + 512] = r[:, 512:1024]
    return out
